# revision 1
# baseline (speedup 1.0000x reference)
"""BERT-with-RoPE attention layer on 8 Trainium2 NeuronCores.

Sharding: core c handles (batch b = c//2, sequence-half hf = c%2).
Each core computes k/v for its batch's full 2048 rows (k/v work duplicated
2x across the pair — cheap) and q + attention + out-projection for its own
1024 query rows, so the 8 output shards are disjoint and the host gather is
a pure concatenation (no collectives).

Matmuls run in bf16 (weights/activations rounded on host or at eviction;
fp32 PSUM accumulate; softmax math in fp32) — the vLLM-standard precision
for this layer. The RoPE halves-swap runs as a float32r PE permutation
matmul so rope arithmetic stays fp32 until the final bf16 rounding.

Device dataflow per core:
  phase A: qT/kT = Wqkv^T @ xT in [outcol, seq] layout (N=1024 bf16 mms),
           bias via ACT eviction, NeoX RoPE as raw*cos + Pswap^T@(raw*sin).
  phase B: v = xT^T @ Wv in natural [seq, dcol] layout, SBUF-resident,
           with a ones column appended per head for softmax sums.
  phase C: per head-pair: scoresT[s2,s1] for both heads as row-tiled
           concurrent matmuls, exp on ACT (scale=1/8 fused, FD=1024),
           ctx^T + sums accumulated in PSUM via the ones column; normalize
           with reciprocal + gpsimd partition-broadcast.
  phase D: outT[Hout,s1] = Wout^T-slices @ ctxT, bias via ACT eviction.
"""

import os
import numpy as np

B, S, H = 4, 2048, 1024
NH, DH = 16, 64
HALF = DH // 2
SQ = S // 2  # query rows per core
KC = H // 128  # hidden contraction chunks
ROPE_BASE = 10000.0
N_CORES = 8

_nc_cache = None
last_results = None


def _build_nc():
    import concourse.bacc as bacc
    import concourse.mybir as mybir
    from concourse.tile import TileContext

    f32 = mybir.dt.float32
    f32r = mybir.dt.float32r
    bf16 = mybir.dt.bfloat16
    Exp = mybir.ActivationFunctionType.Exp
    Ident = mybir.ActivationFunctionType.Identity
    Copy = mybir.ActivationFunctionType.Copy
    MUL = mybir.AluOpType.mult
    ADD = mybir.AluOpType.add

    nc = bacc.Bacc(None, target_bir_lowering=False)

    xT_d = nc.dram_tensor("xT", [KC, 128, S], bf16, kind="ExternalInput")
    wqk_d = nc.dram_tensor("wqk", [16, KC, 128, 128], bf16, kind="ExternalInput")
    wv_d = nc.dram_tensor("wv", [KC, 128, H], bf16, kind="ExternalInput")
    wout_d = nc.dram_tensor("wout", [8, KC, 128, 128], bf16, kind="ExternalInput")
    pswap_d = nc.dram_tensor("pswap", [128, 128], f32r, kind="ExternalInput")
    cosk_d = nc.dram_tensor("cosk", [128, S], f32, kind="ExternalInput")
    sink_d = nc.dram_tensor("sink", [128, S], f32, kind="ExternalInput")
    bqk_d = nc.dram_tensor("bqk", [128, 16], f32, kind="ExternalInput")
    ones_d = nc.dram_tensor("ones", [128, 16], bf16, kind="ExternalInput")
    boutp_d = nc.dram_tensor("boutp", [128, 8], f32, kind="ExternalInput")
    out_d = nc.dram_tensor("outT", [8, 128, SQ], f32, kind="ExternalOutput")
    debug = bool(int(os.environ.get("KERNEL_DEBUG", "0") or "0"))
    if debug:
        dq_d = nc.dram_tensor("dq", [128, KC, SQ], bf16, kind="ExternalOutput")
        dk_d = nc.dram_tensor("dk", [128, KC, S], bf16, kind="ExternalOutput")
        dv_d = nc.dram_tensor("dv", [128, 16, NH, DH + 1], bf16, kind="ExternalOutput")
        dctx_d = nc.dram_tensor("dctx", [128, KC, SQ], bf16, kind="ExternalOutput")

    with TileContext(nc) as tc:
        with (
            tc.tile_pool(name="const", bufs=1) as const,
            tc.tile_pool(name="persist", bufs=1) as persist,
        ):
            pswap_sb = const.tile([128, 128], f32r)
            nc.sync.dma_start(pswap_sb[:, :], pswap_d[:, :])
            bqk_sb = const.tile([128, 16], f32)
            nc.sync.dma_start(bqk_sb[:, :], bqk_d[:, :])
            boutp_sb = const.tile([128, 8], f32)
            nc.sync.dma_start(boutp_sb[:, :], boutp_d[:, :])

            qTr = persist.tile([128, 8, SQ], bf16)
            kTr = persist.tile([128, 8, S], bf16)
            # v resident in SBUF: [s2_in_blk, s2_blk, head, dcol+ones]
            v_sb = persist.tile([128, 16, NH, DH + 1], bf16)

            # ---------------- phase A: q/k projection + rope -------------
            with tc.tile_pool(name="xTp", bufs=1) as xTp:
                xT_sb = xTp.tile([128, KC, S], bf16)
                for c in range(KC):
                    nc.sync.dma_start(xT_sb[:, c, :], xT_d[c, :, :])

                with (
                    tc.tile_pool(name="mapp", bufs=1) as mapp,
                    tc.tile_pool(name="wqkp", bufs=3) as wqkp,
                    tc.tile_pool(name="ropep", bufs=5) as ropep,
                    tc.tile_pool(name="psQK", bufs=2, space="PSUM") as psQK,
                    tc.tile_pool(name="psSW", bufs=2, space="PSUM") as psSW,
                ):
                    cosk_sb = mapp.tile([128, S], f32)
                    nc.sync.dma_start(cosk_sb[:, :], cosk_d[:, :])
                    sink_sb = mapp.tile([128, S], f32)
                    nc.sync.dma_start(sink_sb[:, :], sink_d[:, :])

                    pending = []

                    def _stage2(state):
                        raw, tt, sl, oc = state
                        sw = psSW.tile([128, SQ], f32, tag="sw", name="sw")
                        for hv in range(2):
                            nc.tensor.matmul(
                                sw[:, hv * 512 : (hv + 1) * 512],
                                pswap_sb[:, :], tt[:, hv * 512 : (hv + 1) * 512],
                                start=True, stop=True,
                            )
                        cc = ropep.tile([128, SQ], f32, tag="cc", name="cc")
                        nc.gpsimd.tensor_tensor(
                            cc[:, :], raw[:, :], cosk_sb[:, sl], MUL
                        )
                        if oc < 8:
                            dst = qTr[:, oc, :]
                        else:
                            dst = kTr[:, oc - 8, sl]
                        nc.vector.tensor_tensor(dst, cc[:, :], sw[:, :], ADD)

                    for oc in range(16):
                        wts = []
                        for c in range(KC):
                            wt = wqkp.tile([128, 128], bf16, tag=f"w{c}", name=f"wt{c}")
                            nc.sync.dma_start(wt[:, :], wqk_d[oc, c, :, :])
                            wts.append(wt)
                        nspan = 1 if oc < 8 else 2
                        for sp in range(nspan):
                            sl = slice(sp * SQ, (sp + 1) * SQ)
                            ps = psQK.tile([128, SQ], f32, tag="psQK", name="psQK_t")
                            for c in range(KC):
                                for hv in range(2):
                                    nc.tensor.matmul(
                                        ps[:, hv * 512 : (hv + 1) * 512],
                                        wts[c][:, :],
                                        xT_sb[:, c, sp * SQ + hv * 512 : sp * SQ + (hv + 1) * 512],
                                        start=(c == 0), stop=(c == KC - 1),
                                    )
                            raw = ropep.tile([128, SQ], f32, tag="raw", name="raw")
                            nc.scalar.activation(
                                raw[:, :], ps[:, :], Ident, bias=bqk_sb[:, oc : oc + 1]
                            )
                            tt = ropep.tile([128, SQ], f32r, tag="tt", name="tt")
                            nc.vector.tensor_tensor(
                                tt[:, :], raw[:, :], sink_sb[:, sl], MUL
                            )
                            pending.append((raw, tt, sl, oc))
                            if len(pending) > 1:
                                _stage2(pending.pop(0))
                    while pending:
                        _stage2(pending.pop(0))

                # ------------- phase B: v projection (SBUF-resident) -----
                with (
                    tc.tile_pool(name="wvp", bufs=1) as wvp,
                    tc.tile_pool(name="psV", bufs=3, space="PSUM") as psV,
                ):
                    wvts = []
                    for c in range(KC):
                        wvt = wvp.tile([128, H], bf16, tag=f"wv{c}", name=f"wvt{c}")
                        nc.sync.dma_start(wvt[:, :], wv_d[c, :, :])
                        wvts.append(wvt)
                    for sb in range(16):
                        ps = psV.tile([128, H], f32, tag="psV", name="psV_t")
                        for c in range(KC):
                            for hv in range(2):
                                nc.tensor.matmul(
                                    ps[:, hv * 512 : (hv + 1) * 512],
                                    xT_sb[:, c, sb * 128 : (sb + 1) * 128],
                                    wvts[c][:, hv * 512 : (hv + 1) * 512],
                                    start=(c == 0), stop=(c == KC - 1),
                                )
                        nc.scalar.activation(
                            v_sb[:, sb, :, 0:DH],
                            ps.rearrange("p (h d) -> p h d", h=NH),
                            Copy,
                        )
                        nc.sync.dma_start(v_sb[:, sb, :, DH : DH + 1], ones_d[:, :])

            # ---------------- phase C: attention (head pairs) ------------
            ctxp = tc.alloc_tile_pool(name="ctxp", bufs=1)
            ctxT = ctxp.tile([128, KC, SQ], bf16)
            with (
                tc.tile_pool(name="expp", bufs=6) as expp,
                tc.tile_pool(name="scrp", bufs=4) as scrp,
                tc.tile_pool(name="psSc", bufs=3, space="PSUM") as psSc,
                tc.tile_pool(name="psCtx", bufs=1, space="PSUM") as psCtx,
            ):
                for pr in range(8):
                    for hf in range(2):
                        s1 = slice(hf * 512, (hf + 1) * 512)
                        cE = psCtx.tile([128, 512], f32, tag="ctxe", name="cE")
                        cO = psCtx.tile([128, 512], f32, tag="ctxo", name="cO")
                        for blk in range(16):
                            sc = psSc.tile([128, 2, 512], f32, tag="sc", name="sc")
                            for par in range(2):
                                rs = par * 64
                                nc.tensor.matmul(
                                    sc[:, par, :],
                                    kTr[rs : rs + 64, pr, blk * 128 : (blk + 1) * 128],
                                    qTr[rs : rs + 64, pr, s1],
                                    start=True, stop=True,
                                )
                            et = expp.tile([128, 2, 512], bf16, tag="et", name="et")
                            nc.scalar.activation(
                                et[:, :, :], sc[:, :, :], Exp, scale=0.125
                            )
                            st, sp_ = (blk == 0), (blk == 15)
                            nc.tensor.matmul(
                                cE[0 : DH + 1, :],
                                v_sb[:, blk, 2 * pr, :], et[:, 0, :],
                                start=st, stop=sp_,
                            )
                            nc.tensor.matmul(
                                cO[0 : DH + 1, :],
                                v_sb[:, blk, 2 * pr + 1, :], et[:, 1, :],
                                start=st, stop=sp_,
                            )
                        # epilogue per (pair, s1-half): normalize ctx rows
                        # 0..63 by sums row 64. reciprocal_approx_fast /
                        # partition_broadcast act on tensor partition 0
                        # regardless of AP base, so route the sums row
                        # through partition 0 via DMA.
                        for par, ct in ((0, cE), (1, cO)):
                            scr = scrp.tile([128, 512], f32, tag="scr", name="scr")
                            nc.vector.tensor_copy(scr[64:65, :], ct[64:65, :])
                            scr2 = scrp.tile([1, 512], f32, tag="scr2", name="scr2")
                            nc.sync.dma_start(scr2[0:1, :], scr[64:65, :])
                            bcs = scrp.tile([128, 512], f32, tag="bcs", name="bcs")
                            nc.gpsimd.partition_broadcast(bcs[0:64, :], scr2[0:1, :])
                            bc = scrp.tile([128, 512], f32, tag="bc", name="bc")
                            nc.vector.reciprocal_approx_fast(bc[0:64, :], bcs[0:64, :])
                            if par == 0:
                                nc.vector.tensor_tensor(
                                    ctxT[0:64, pr, s1], ct[0:64, :], bc[0:64, :], MUL
                                )
                            else:
                                tmp = scrp.tile([64, 512], bf16, tag="tmp", name="tmp")
                                nc.vector.tensor_tensor(
                                    tmp[:, :], ct[0:64, :], bc[0:64, :], MUL
                                )
                                nc.sync.dma_start(ctxT[64:128, pr, s1], tmp[:, :])

            if debug:
                nc.sync.dma_start(dq_d[:, :, :], qTr[:, :, :])
                nc.sync.dma_start(dk_d[:, :, :], kTr[:, :, :])
                nc.sync.dma_start(dv_d[:, :, :, :], v_sb[:, :, :, :])
                nc.sync.dma_start(dctx_d[:, :, :], ctxT[:, :, :])

            # ---------------- phase D: out projection -------------------
            with (
                tc.tile_pool(name="woutp", bufs=3) as woutp,
                tc.tile_pool(name="obp", bufs=3) as obp,
                tc.tile_pool(name="psO", bufs=2, space="PSUM") as psO,
            ):
                for hb in range(8):
                    owts = []
                    for c in range(KC):
                        owt = woutp.tile([128, 128], bf16, tag=f"o{c}", name=f"owt{c}")
                        nc.sync.dma_start(owt[:, :], wout_d[hb, c, :, :])
                        owts.append(owt)
                    ps = psO.tile([128, SQ], f32, tag="psO", name="psO_t")
                    for c in range(KC):
                        for hv in range(2):
                            nc.tensor.matmul(
                                ps[:, hv * 512 : (hv + 1) * 512],
                                owts[c][:, :],
                                ctxT[:, c, hv * 512 : (hv + 1) * 512],
                                start=(c == 0), stop=(c == KC - 1),
                            )
                    ob = obp.tile([128, SQ], f32, tag="ob", name="ob")
                    nc.scalar.activation(
                        ob[:, :], ps[:, :], Ident, bias=boutp_sb[:, hb : hb + 1]
                    )
                    nc.sync.dma_start(out_d[hb, :, :], ob[:, :])
            ctxp.release()

    nc.finalize()
    return nc


def _host_prep(positions, hidden_states, Wqkv, bqkv, Wout, bout):
    import ml_dtypes

    bf16 = ml_dtypes.bfloat16
    positions = np.asarray(positions)
    hidden_states = np.asarray(hidden_states, dtype=np.float32)
    Wqkv = np.asarray(Wqkv, dtype=np.float32)
    bqkv = np.asarray(bqkv, dtype=np.float32)
    Wout = np.asarray(Wout, dtype=np.float32)
    bout = np.asarray(bout, dtype=np.float32)

    wqk = np.ascontiguousarray(
        Wqkv[:, : 2 * H].reshape(KC, 128, 16, 128).transpose(2, 0, 1, 3)
    ).astype(bf16)
    wv = np.ascontiguousarray(Wqkv[:, 2 * H :].reshape(KC, 128, H)).astype(bf16)
    wout_t = np.ascontiguousarray(
        Wout.reshape(KC, 128, 8, 128).transpose(2, 0, 1, 3)
    ).astype(bf16)
    bqk = np.ascontiguousarray(bqkv[: 2 * H].reshape(16, 128).T)
    boutp_full = bout.astype(np.float64) + bqkv[2 * H :].astype(
        np.float64
    ) @ Wout.astype(np.float64)
    boutp = np.ascontiguousarray(boutp_full.astype(np.float32).reshape(8, 128).T)

    pswap = np.zeros((128, 128), dtype=np.float32)
    for m in range(128):
        if m % 64 < HALF:
            pswap[m + HALF, m] = -1.0
        else:
            pswap[m - HALF, m] = 1.0

    inv_freq = 1.0 / (ROPE_BASE ** (np.arange(HALF, dtype=np.float64) / HALF))
    rowmap = np.arange(128) % HALF

    in_maps = []
    for c in range(N_CORES):
        b, hf = c // 2, c % 2
        perm = np.concatenate(
            [np.arange(hf * SQ, (hf + 1) * SQ), np.arange((1 - hf) * SQ, (2 - hf) * SQ)]
        )
        x_perm = hidden_states[b][perm]
        xT = np.ascontiguousarray(x_perm.T).reshape(KC, 128, S).astype(bf16)
        pos = positions[perm].astype(np.float64)
        freqs = pos[:, None] * inv_freq[None, :]  # [S, HALF]
        cosk = np.ascontiguousarray(np.cos(freqs).astype(np.float32)[:, rowmap].T)
        sink = np.ascontiguousarray(np.sin(freqs).astype(np.float32)[:, rowmap].T)
        in_maps.append(
            {
                "xT": xT, "wqk": wqk, "wv": wv, "wout": wout_t,
                "pswap": pswap, "cosk": cosk, "sink": sink,
                "bqk": bqk, "boutp": boutp,
                "ones": np.ones((128, 16), dtype=bf16),
            }
        )
    return in_maps


def kernel(positions, hidden_states, Wqkv, bqkv, Wout, bout):
    global _nc_cache, last_results
    from concourse import bass_utils

    if _nc_cache is None:
        _nc_cache = _build_nc()
    nc = _nc_cache

    in_maps = _host_prep(positions, hidden_states, Wqkv, bqkv, Wout, bout)
    res = bass_utils.run_bass_kernel_spmd(
        nc, in_maps, core_ids=list(range(N_CORES)),
        trace=bool(int(os.environ.get("KERNEL_TRACE", "0") or "0")),
    )
    last_results = res

    out = np.empty((B, S, H), dtype=np.float32)
    for c in range(N_CORES):
        b, hf = c // 2, c % 2
        outT = np.asarray(res.results[c]["outT"]).reshape(H, SQ)
        out[b, hf * SQ : (hf + 1) * SQ, :] = outT.T
    return out



# revision 5
# speedup vs baseline: 1.0895x; 1.0895x over previous
"""BERT-with-RoPE attention layer on 8 Trainium2 NeuronCores.

Sharding: core c handles (batch b = c//2, head-half hh = c%2): 8 of the 16
heads over the FULL 2048-token sequence. q/k/v are computed only for the
core's own heads (no duplicated work); Wout is row-sharded (vLLM style), so
each core emits a full-shape PARTIAL output and the all-reduce degenerates
to a host-side sum of the two partials per batch (free - not on device).

Single fused device pipeline per core:
  The attention stream (scores -> exp -> ctx) is ACT-engine bound (~1us per
  key-block for the 1024-elem exp).  All other matmul work - q/k projection
  chains with RoPE, v projection groups, out-projection groups - is emitted
  as PE "filler" between attention blocks so the tensor engine computes
  underneath the exp stream instead of in separate phases.  This also keeps
  the PE HAM clock-gate warm (no idle windows > 3.4us).

  PSUM budget (8 banks): scores double-buffered (2 tiles x 2 banks), ctx
  accumulators cE/cO (2 banks, single-buffered - freed fast via DVE copy to
  SBUF right after the last ctx matmul), shared 2-bank spare pool for all
  filler accumulations (projection chains, rope-swap, v, out-proj).

  Matmuls in bf16 (fp32 PSUM accumulate); softmax in fp32 via ACT exp with
  fused 1/8 scale; NeoX RoPE halves-swap as a float32r PE permutation
  matmul; softmax sums via a ones column appended to v (65th PSUM row).
"""

import os
import numpy as np
from collections import deque

B, S, H = 4, 2048, 1024
NH, DH = 16, 64
HALF = DH // 2
KC = H // 128        # x contraction chunks
NOC = 8              # q/k output col chunks per core (0-3 q, 4-7 k)
PRS = 4              # head pairs per core
NQS = 4              # 512-col query spans
KCD = 4              # out-proj contraction chunks (512 rows / 128)
ROPE_BASE = 10000.0
N_CORES = 8

_nc_cache = None
last_results = None


def _build_nc():
    import concourse.bacc as bacc
    import concourse.mybir as mybir
    from concourse.tile import TileContext

    f32 = mybir.dt.float32
    f32r = mybir.dt.float32r
    bf16 = mybir.dt.bfloat16
    Exp = mybir.ActivationFunctionType.Exp
    MUL = mybir.AluOpType.mult
    ADD = mybir.AluOpType.add

    nc = bacc.Bacc(None, target_bir_lowering=False)

    xT_d = nc.dram_tensor("xT", [KC, 128, S], bf16, kind="ExternalInput")
    wqk_d = nc.dram_tensor("wqk", [NOC, 128, KC, 128], bf16, kind="ExternalInput")
    wv_d = nc.dram_tensor("wv", [128, KC, 512], bf16, kind="ExternalInput")
    wout_d = nc.dram_tensor("wout", [8, 128, KCD, 128], bf16, kind="ExternalInput")
    pswap_d = nc.dram_tensor("pswap", [128, 128], f32r, kind="ExternalInput")
    cosk_d = nc.dram_tensor("cosk", [128, S], f32, kind="ExternalInput")
    sink_d = nc.dram_tensor("sink", [128, S], f32, kind="ExternalInput")
    bqk_d = nc.dram_tensor("bqk", [128, NOC], f32, kind="ExternalInput")
    ones_d = nc.dram_tensor("ones", [128, 8], bf16, kind="ExternalInput")
    out_d = nc.dram_tensor("outT", [8, 128, S], bf16, kind="ExternalOutput")
    debug = bool(int(os.environ.get("KERNEL_DEBUG", "0") or "0"))
    if debug:
        dq_d = nc.dram_tensor("dq", [128, PRS, S], bf16, kind="ExternalOutput")
        dk_d = nc.dram_tensor("dk", [128, PRS, S], bf16, kind="ExternalOutput")
        dv_d = nc.dram_tensor("dv", [128, 16, 8, DH + 1], bf16, kind="ExternalOutput")
        dctx_d = nc.dram_tensor("dctx", [128, PRS, S], bf16, kind="ExternalOutput")

    with TileContext(nc) as tc:
        with (
            tc.tile_pool(name="const", bufs=1) as const,
            tc.tile_pool(name="persist", bufs=1) as persist,
            tc.tile_pool(name="wqkp", bufs=3) as wqkp,
            tc.tile_pool(name="ropep", bufs=3) as ropep,
            tc.tile_pool(name="expp", bufs=3) as expp,
            tc.tile_pool(name="scrp", bufs=2) as scrp,
            tc.tile_pool(name="obp", bufs=2) as obp,
            tc.tile_pool(name="spare", bufs=2, space="PSUM") as spare,
            tc.tile_pool(name="psSc", bufs=2, space="PSUM") as psSc,
            tc.tile_pool(name="psCtx", bufs=1, space="PSUM") as psCtx,
        ):
            pswap_sb = const.tile([128, 128], f32r)
            nc.sync.dma_start(pswap_sb[:, :], pswap_d[:, :])
            bqk_sb = const.tile([128, NOC], f32)
            nc.sync.dma_start(bqk_sb[:, :], bqk_d[:, :])
            ones_sb = const.tile([128, 8], bf16)
            nc.sync.dma_start(ones_sb[:, :], ones_d[:, :])

            xT_sb = persist.tile([128, KC, S], bf16)
            cosk_sb = persist.tile([128, S], f32)
            sink_sb = persist.tile([128, S], f32)
            qTr = persist.tile([128, PRS, S], bf16)
            kTr = persist.tile([128, PRS, S], bf16)
            v_sb = persist.tile([128, 16, 8, DH + 1], bf16)
            ctxT = persist.tile([128, PRS, S], bf16)
            wv_sb = persist.tile([128, KC, 512], bf16)
            wout_sb = [persist.tile([128, KCD, 128], bf16, name=f"wo{o}")
                       for o in range(8)]

            # ---------------- DMA emission (single FIFO queue: order by
            # consumption).  k(p0)+q(p0) weights, then xT quarter-by-quarter
            # with cos/sin, wv after the first quarter. ----------------
            wqk_tiles = {}

            def load_wqk(oc):
                if oc not in wqk_tiles:
                    w = wqkp.tile([128, KC, 128], bf16, tag="w", name=f"wqk{oc}")
                    nc.sync.dma_start(w[:, :, :], wqk_d[oc, :, :, :])
                    wqk_tiles[oc] = w
                return wqk_tiles[oc]

            # ones column of v (tiny; must precede the big streams on the
            # single FIFO DMA queue - first ctx matmul needs it)
            for sb in range(16):
                nc.sync.dma_start(v_sb[:, sb, :, DH:DH + 1], ones_sb[:, :])
            load_wqk(4)  # k chunk 0 (pair 0)
            load_wqk(0)  # q chunk 0 (pair 0)
            for qs in range(NQS):
                sl = slice(qs * 512, (qs + 1) * 512)
                for c in range(KC):
                    nc.sync.dma_start(xT_sb[:, c, sl], xT_d[c, :, sl])
                nc.sync.dma_start(sink_sb[:, sl], sink_d[:, sl])
                nc.sync.dma_start(cosk_sb[:, sl], cosk_d[:, sl])
                if qs == 0:
                    nc.sync.dma_start(wv_sb[:, :, :], wv_d[:, :, :])

            # ---------------- building blocks ----------------
            def chain_gen(oc, qs):
                """q/k projection chain for one 512-col span, with RoPE."""
                w = load_wqk(oc)
                sl = slice(qs * 512, (qs + 1) * 512)
                ps = spare.tile([128, 512], f32, tag="ps", name="psqk")
                for c in range(KC):
                    nc.tensor.matmul(
                        ps[:, :], w[:, c, :], xT_sb[:, c, sl],
                        start=(c == 0), stop=(c == KC - 1),
                    )
                    if c == 3:
                        yield
                raw = ropep.tile([128, 512], f32, tag="raw", name="raw")
                nc.vector.tensor_scalar_add(raw[:, :], ps[:, :],
                                            bqk_sb[:, oc:oc + 1])
                tt = ropep.tile([128, 512], f32r, tag="tt", name="tt")
                nc.vector.tensor_tensor(tt[:, :], raw[:, :], sink_sb[:, sl], MUL)
                yield
                sw = spare.tile([128, 512], f32, tag="ps", name="pssw")
                nc.tensor.matmul(sw[:, :], pswap_sb[:, :], tt[:, :],
                                 start=True, stop=True)
                cc = ropep.tile([128, 512], f32, tag="cc", name="cc")
                nc.gpsimd.tensor_tensor(cc[:, :], raw[:, :], cosk_sb[:, sl], MUL)
                dst = qTr[:, oc, sl] if oc < 4 else kTr[:, oc - 4, sl]
                nc.vector.tensor_tensor(dst, cc[:, :], sw[:, :], ADD)
                yield

            def vgroup_p0(sb):
                """v projection for head pair 0 (128 cols) of seq block sb."""
                sbc = slice(sb * 128, (sb + 1) * 128)
                ps = spare.tile([128, 512], f32, tag="ps", name="psv0")
                for c in range(KC):
                    nc.tensor.matmul(
                        ps[:, 0:128], xT_sb[:, c, sbc], wv_sb[:, c, 0:128],
                        start=(c == 0), stop=(c == KC - 1),
                    )
                nc.vector.tensor_copy(
                    v_sb[:, sb, 0:2, 0:DH],
                    ps[:, 0:128].rearrange("p (h d) -> p h d", h=2),
                )

            def vrest_gen(sb):
                """v projection for head pairs 1-3 (384 cols) of block sb."""
                sbc = slice(sb * 128, (sb + 1) * 128)
                ps = spare.tile([128, 512], f32, tag="ps", name="psvr")
                for c in range(KC):
                    nc.tensor.matmul(
                        ps[:, 0:384], xT_sb[:, c, sbc], wv_sb[:, c, 128:512],
                        start=(c == 0), stop=(c == KC - 1),
                    )
                    if c == 3:
                        yield
                nc.vector.tensor_copy(
                    v_sb[:, sb, 2:8, 0:DH],
                    ps[:, 0:384].rearrange("p (h d) -> p h d", h=6),
                )
                yield

            def wout_load_gen():
                for o in range(8):
                    nc.sync.dma_start(wout_sb[o][:, :, :], wout_d[o, :, :, :])
                yield

            def dgroup_gen(oc, qs):
                """out-projection for one (col chunk, 512-col q span)."""
                sl = slice(qs * 512, (qs + 1) * 512)
                ps = spare.tile([128, 512], f32, tag="ps", name="pso")
                for c in range(KCD):
                    nc.tensor.matmul(
                        ps[:, :], wout_sb[oc][:, c, :], ctxT[:, c, sl],
                        start=(c == 0), stop=(c == KCD - 1),
                    )
                yield
                ob = obp.tile([128, 512], bf16, tag="ob", name="ob")
                nc.vector.tensor_copy(ob[:, :], ps[:, :])
                nc.sync.dma_start(out_d[oc, :, sl], ob[:, :])
                yield

            # ---------------- filler schedule ----------------
            # (ready, deadline, generator), queue sorted by deadline.
            # pump() advances the head between attention blocks but never
            # starts an item before `ready` (so a far-future dep can't park
            # in the in-order engine queues).  drain_due() force-emits every
            # item whose deadline has arrived BEFORE the consuming iteration
            # is emitted - program order is semantic order in Tile, so a
            # consumer emitted before its producer would read garbage.
            fillers = deque()
            for qs in range(1, NQS):          # q(p0) spans 1-3
                fillers.append((0, qs, chain_gen(0, qs)))
            for sb in range(16):              # v pairs 1-3: before iter 4
                fillers.append((1, 4, vrest_gen(sb)))
            for qs in range(NQS):             # k(p1)
                fillers.append((1, 4, chain_gen(5, qs)))
            for qs in range(NQS):             # q(p1)
                fillers.append((max(1, qs), 4 + qs, chain_gen(1, qs)))
            for qs in range(NQS):             # k(p2)
                fillers.append((4, 8, chain_gen(6, qs)))
            for qs in range(NQS):             # q(p2)
                fillers.append((4 + min(qs, 1), 8 + qs, chain_gen(2, qs)))
            for qs in range(NQS):             # k(p3)
                fillers.append((8, 12, chain_gen(7, qs)))
            for qs in range(NQS):             # q(p3)
                fillers.append((8 + min(qs, 1), 12 + qs, chain_gen(3, qs)))
            fillers.append((8, 13, wout_load_gen()))
            for qs in range(NQS):             # out-proj after C(p3, sp=qs)
                for oc in range(8):
                    fillers.append((13 + qs, 99, dgroup_gen(oc, qs)))

            cur_iter = [0]

            def pump(n):
                while n > 0 and fillers:
                    ready, _, gen = fillers[0]
                    if ready > cur_iter[0]:
                        return
                    try:
                        next(gen)
                        n -= 1
                    except StopIteration:
                        fillers.popleft()

            def drain_due(it):
                while fillers and fillers[0][1] <= it:
                    _, _, gen = fillers[0]
                    for _ in gen:
                        pass
                    fillers.popleft()

            # ---------------- preamble: k(p0) span 0 + q(p0) span 0; the
            # remaining k(p0) spans are emitted inline in iteration 0 right
            # before the score blocks that consume them, pacing with the
            # xT DMA stream instead of serializing the full load up front.
            for _ in chain_gen(4, 0):
                pass
            for _ in chain_gen(0, 0):
                pass

            # ---------------- fused attention stream ----------------
            for pr in range(PRS):
                for sp in range(NQS):
                    it = pr * NQS + sp
                    cur_iter[0] = it
                    drain_due(it)
                    s1 = slice(sp * 512, (sp + 1) * 512)
                    cE = psCtx.tile([128, 512], f32, tag="ce", name="cE")
                    cO = psCtx.tile([128, 512], f32, tag="co", name="cO")
                    for blk in range(16):
                        if it == 0 and blk in (4, 8, 12):
                            for _ in chain_gen(4, blk // 4):
                                pass
                        sc = psSc.tile([128, 2, 512], f32, tag="sc", name="sc")
                        for par in range(2):
                            rs = par * 64
                            nc.tensor.matmul(
                                sc[:, par, :],
                                kTr[rs:rs + 64, pr, blk * 128:(blk + 1) * 128],
                                qTr[rs:rs + 64, pr, s1],
                                start=True, stop=True,
                            )
                        et = expp.tile([128, 2, 512], bf16, tag="et", name="et")
                        nc.scalar.activation(et[:, :, :], sc[:, :, :], Exp,
                                             scale=0.125)
                        if it == 0:
                            vgroup_p0(blk)
                        else:
                            pump(1)
                        st, sp_ = (blk == 0), (blk == 15)
                        nc.tensor.matmul(
                            cE[0:DH + 1, :], v_sb[:, blk, 2 * pr, :],
                            et[:, 0, :], start=st, stop=sp_,
                        )
                        nc.tensor.matmul(
                            cO[0:DH + 1, :], v_sb[:, blk, 2 * pr + 1, :],
                            et[:, 1, :], start=st, stop=sp_,
                        )
                    # epilogue part 1: drain ctx accumulators to SBUF fast
                    # so the single-buffered PSUM banks free immediately.
                    ctf = scrp.tile([128, 2, 512], f32, tag="ctf", name="ctf")
                    nc.vector.tensor_copy(ctf[0:DH + 1, 0, :], cE[0:DH + 1, :])
                    nc.vector.tensor_copy(ctf[0:DH + 1, 1, :], cO[0:DH + 1, :])
                    # sums row (partition 64) -> partition 0 via DMA: the
                    # broadcast/recip ops act on tensor partition 0 only.
                    scr2 = scrp.tile([1, 2, 512], f32, tag="scr2", name="scr2")
                    nc.sync.dma_start(scr2[0:1, :, :], ctf[DH:DH + 1, :, :])
                    pump(2)
                    # epilogue part 2: normalize off the critical path.
                    bcs = scrp.tile([64, 2, 512], f32, tag="bcs", name="bcs")
                    nc.gpsimd.partition_broadcast(bcs[0:DH, :, :],
                                                  scr2[0:1, :, :])
                    bc = scrp.tile([64, 2, 512], f32, tag="bc", name="bc")
                    nc.vector.reciprocal_approx_fast(bc[0:DH, :, :],
                                                     bcs[0:DH, :, :])
                    nc.vector.tensor_tensor(ctxT[0:DH, pr, s1],
                                            ctf[0:DH, 0, :], bc[0:DH, 0, :],
                                            MUL)
                    tmp = scrp.tile([64, 512], bf16, tag="tmp", name="tmp")
                    nc.vector.tensor_tensor(tmp[:, :], ctf[0:DH, 1, :],
                                            bc[0:DH, 1, :], MUL)
                    nc.sync.dma_start(ctxT[DH:128, pr, s1], tmp[:, :])
                    pump(2)

            # ---------------- drain remaining fillers ----------------
            cur_iter[0] = 99
            while fillers:
                pump(100)

            if debug:
                nc.sync.dma_start(dq_d[:, :, :], qTr[:, :, :])
                nc.sync.dma_start(dk_d[:, :, :], kTr[:, :, :])
                nc.sync.dma_start(dv_d[:, :, :, :], v_sb[:, :, :, :])
                nc.sync.dma_start(dctx_d[:, :, :], ctxT[:, :, :])

    nc.finalize()
    return nc


def _host_prep(positions, hidden_states, Wqkv, bqkv, Wout, bout):
    import ml_dtypes

    bf16 = ml_dtypes.bfloat16
    hidden_states = np.asarray(hidden_states, dtype=np.float32)
    Wqkv = np.asarray(Wqkv, dtype=np.float32)
    bqkv = np.asarray(bqkv, dtype=np.float32)
    Wout = np.asarray(Wout, dtype=np.float32)
    positions = np.asarray(positions)

    pswap = np.zeros((128, 128), dtype=np.float32)
    for m in range(128):
        if m % DH < HALF:
            pswap[m + HALF, m] = -1.0
        else:
            pswap[m - HALF, m] = 1.0

    inv_freq = 1.0 / (ROPE_BASE ** (np.arange(HALF, dtype=np.float64) / HALF))
    rowmap = np.arange(128) % HALF
    freqs = positions.astype(np.float64)[:, None] * inv_freq[None, :]  # [S, 32]
    cosk = np.ascontiguousarray(np.cos(freqs).astype(np.float32)[:, rowmap].T)
    sink = np.ascontiguousarray(np.sin(freqs).astype(np.float32)[:, rowmap].T)
    ones = np.ones((128, 8), dtype=bf16)

    xTs = []
    for b in range(B):
        xTs.append(np.ascontiguousarray(
            hidden_states[b].T).reshape(KC, 128, S).astype(bf16))

    per_hh = []
    for hh in range(2):
        qsl = slice(hh * 512, (hh + 1) * 512)
        ksl = slice(H + hh * 512, H + (hh + 1) * 512)
        vsl = slice(2 * H + hh * 512, 2 * H + (hh + 1) * 512)
        wq = np.ascontiguousarray(
            Wqkv[:, qsl].reshape(KC, 128, 4, 128).transpose(2, 1, 0, 3))
        wk = np.ascontiguousarray(
            Wqkv[:, ksl].reshape(KC, 128, 4, 128).transpose(2, 1, 0, 3))
        wqk = np.concatenate([wq, wk], axis=0).astype(bf16)  # [8,128,KC,128]
        wv = np.ascontiguousarray(
            Wqkv[:, vsl].reshape(KC, 128, 512).transpose(1, 0, 2)).astype(bf16)
        wout = np.ascontiguousarray(
            Wout[hh * 512:(hh + 1) * 512, :]
            .reshape(KCD, 128, 8, 128).transpose(2, 1, 0, 3)).astype(bf16)
        bq = bqkv[:H][qsl].reshape(4, 128).T
        bk = bqkv[H:2 * H][hh * 512:(hh + 1) * 512].reshape(4, 128).T
        bqk = np.ascontiguousarray(np.concatenate([bq, bk], axis=1))  # [128,8]
        per_hh.append((wqk, wv, wout, bqk))

    in_maps = []
    for c in range(N_CORES):
        b, hh = c // 2, c % 2
        wqk, wv, wout, bqk = per_hh[hh]
        in_maps.append({
            "xT": xTs[b], "wqk": wqk, "wv": wv, "wout": wout,
            "pswap": pswap, "cosk": cosk, "sink": sink,
            "bqk": bqk, "ones": ones,
        })
    return in_maps


def kernel(positions, hidden_states, Wqkv, bqkv, Wout, bout):
    global _nc_cache, last_results
    from concourse import bass_utils

    if _nc_cache is None:
        _nc_cache = _build_nc()
    nc = _nc_cache

    in_maps = _host_prep(positions, hidden_states, Wqkv, bqkv, Wout, bout)
    res = bass_utils.run_bass_kernel_spmd(
        nc, in_maps, core_ids=list(range(N_CORES)),
        trace=bool(int(os.environ.get("KERNEL_TRACE", "0") or "0")),
    )
    last_results = res

    bqkv = np.asarray(bqkv, dtype=np.float32)
    Wout = np.asarray(Wout, dtype=np.float32)
    bout = np.asarray(bout, dtype=np.float32)
    # v-bias contribution (attn rows sum to 1) + output bias, added on host
    bias_full = (bout + bqkv[2 * H:].astype(np.float64) @
                 Wout.astype(np.float64)).astype(np.float32)

    out = np.empty((B, S, H), dtype=np.float32)
    for b in range(B):
        p0 = np.asarray(res.results[2 * b]["outT"]).astype(np.float32)
        p1 = np.asarray(res.results[2 * b + 1]["outT"]).astype(np.float32)
        o = (p0 + p1).reshape(H, S)
        out[b] = o.T + bias_full[None, :]
    return out


# revision 19
# speedup vs baseline: 1.2586x; 1.1552x over previous
"""BERT-with-RoPE attention layer on 8 Trainium2 NeuronCores.

Sharding: core c handles (batch b = c//2, head-half hh = c%2): 8 of the 16
heads over the FULL 2048-token sequence. q/k/v are computed only for the
core's own heads (no duplicated work); Wout is row-sharded (vLLM style), so
each core emits a full-shape PARTIAL output and the all-reduce degenerates
to a host-side sum of the two partials per batch (free - not on device).

Single fused device pipeline per core:
  The attention stream (scores -> exp -> ctx) is ACT-engine bound (~1us per
  key-block for the 1024-elem exp).  All other matmul work - q/k projection
  chains with RoPE, v projection groups, out-projection groups - is emitted
  as PE "filler" between attention blocks so the tensor engine computes
  underneath the exp stream instead of in separate phases.  This also keeps
  the PE HAM clock-gate warm (no idle windows > 3.4us).

  PSUM budget (8 banks): scores double-buffered (2 tiles x 2 banks), ctx
  accumulators cE/cO (2 banks, single-buffered - freed fast via DVE copy to
  SBUF right after the last ctx matmul), shared 2-bank spare pool for all
  filler accumulations (projection chains, rope-swap, v, out-proj).

  Matmuls in bf16 (fp32 PSUM accumulate); softmax in fp32 via ACT exp with
  fused 1/8 scale; NeoX RoPE halves-swap as a float32r PE permutation
  matmul; softmax sums via a ones column appended to v (65th PSUM row).
"""

import os
import numpy as np
from collections import deque

B, S, H = 4, 2048, 1024
NH, DH = 16, 64
HALF = DH // 2
KC = H // 128        # x contraction chunks
NOC = 8              # q/k output col chunks per core (0-3 q, 4-7 k)
PRS = 4              # head pairs per core
NQS = 4              # 512-col query spans
KCD = 4              # out-proj contraction chunks (512 rows / 128)
ROPE_BASE = 10000.0
N_CORES = 8

_nc_cache = None
last_results = None


def _build_nc():
    import concourse.bacc as bacc
    import concourse.mybir as mybir
    from concourse.tile import TileContext

    f32 = mybir.dt.float32
    f32r = mybir.dt.float32r
    bf16 = mybir.dt.bfloat16
    Exp = mybir.ActivationFunctionType.Exp
    MUL = mybir.AluOpType.mult
    ADD = mybir.AluOpType.add

    nc = bacc.Bacc(None, target_bir_lowering=False)

    xT_d = nc.dram_tensor("xT", [128, KC, S], bf16, kind="ExternalInput")
    wqk_d = nc.dram_tensor("wqk", [NOC, 128, KC, 128], bf16, kind="ExternalInput")
    wv_d = nc.dram_tensor("wv", [128, KC, 512], bf16, kind="ExternalInput")
    wout_d = nc.dram_tensor("wout", [128, 8, KCD, 128], bf16, kind="ExternalInput")
    pswap_d = nc.dram_tensor("pswap", [128, 128], f32r, kind="ExternalInput")
    cosk_d = nc.dram_tensor("cosk", [128, S], f32, kind="ExternalInput")
    sink_d = nc.dram_tensor("sink", [128, S], f32, kind="ExternalInput")
    bqk_d = nc.dram_tensor("bqk", [128, NOC], f32, kind="ExternalInput")
    out_d = nc.dram_tensor("outT", [8, 128, S], bf16, kind="ExternalOutput")
    debug = bool(int(os.environ.get("KERNEL_DEBUG", "0") or "0"))
    if debug:
        dq_d = nc.dram_tensor("dq", [128, PRS, S], bf16, kind="ExternalOutput")
        dk_d = nc.dram_tensor("dk", [128, PRS, S], bf16, kind="ExternalOutput")
        dv_d = nc.dram_tensor("dv", [128, 16, 8, DH + 1], bf16, kind="ExternalOutput")
        dctx_d = nc.dram_tensor("dctx", [128, PRS, S], bf16, kind="ExternalOutput")

    with TileContext(nc) as tc:
        with (
            tc.tile_pool(name="const", bufs=1) as const,
            tc.tile_pool(name="persist", bufs=1) as persist,
            tc.tile_pool(name="wqkp", bufs=3) as wqkp,
            tc.tile_pool(name="ropep", bufs=3) as ropep,
            tc.tile_pool(name="expp", bufs=3) as expp,
            tc.tile_pool(name="scrp", bufs=2) as scrp,
            tc.tile_pool(name="obp", bufs=2) as obp,
            tc.tile_pool(name="spare", bufs=2, space="PSUM") as spare,
            tc.tile_pool(name="psSc", bufs=2, space="PSUM") as psSc,
            tc.tile_pool(name="psCtx", bufs=1, space="PSUM") as psCtx,
        ):
            pswap_sb = const.tile([128, 128], f32r)
            nc.sync.dma_start(pswap_sb[:, :], pswap_d[:, :])
            bqk_sb = const.tile([128, NOC], f32)
            nc.sync.dma_start(bqk_sb[:, :], bqk_d[:, :])

            xT_sb = persist.tile([128, KC, S], bf16)
            cosk_sb = persist.tile([128, S], f32)
            sink_sb = persist.tile([128, S], f32)
            qTr = persist.tile([128, PRS, S], bf16)
            kTr = persist.tile([128, PRS, S], bf16)
            v_sb = persist.tile([128, 16, 8, DH + 1], bf16)
            ctxT = persist.tile([128, PRS, S], bf16)
            wv_sb = persist.tile([128, KC, 512], bf16)
            wout_sb = persist.tile([128, 8, KCD, 128], bf16)

            # ones column of v via memset - scattered 2-byte DMAs would
            # serialize ~10us of Sync-engine issue time at the queue head
            nc.gpsimd.memset(v_sb[:, :, :, DH:DH + 1], 1.0)

            # ---------------- DMA emission (single FIFO queue: order by
            # consumption; each dma_start costs ~0.65us of Sync issue, so
            # few big DMAs).  k(p0)+q(p0) weights, then xT quarter-by-
            # quarter with sin/cos, wv after the first quarter. ---------
            wqk_tiles = {}

            def load_wqk(oc):
                if oc not in wqk_tiles:
                    w = wqkp.tile([128, KC, 128], bf16, tag="w", name=f"wqk{oc}")
                    nc.sync.dma_start(w[:, :, :], wqk_d[oc, :, :, :])
                    wqk_tiles[oc] = w
                return wqk_tiles[oc]

            load_wqk(4)  # k chunk 0 (pair 0)
            load_wqk(0)  # q chunk 0 (pair 0)
            for qs in range(NQS):
                sl = slice(qs * 512, (qs + 1) * 512)
                nc.sync.dma_start(xT_sb[:, :, sl], xT_d[:, :, sl])
                nc.sync.dma_start(sink_sb[:, sl], sink_d[:, sl])
                nc.sync.dma_start(cosk_sb[:, sl], cosk_d[:, sl])
                if qs == 0:
                    nc.sync.dma_start(wv_sb[:, :, :], wv_d[:, :, :])

            # ---------------- building blocks ----------------
            # q/k chains are split in two stages, software-pipelined one
            # chain deep: stage 2 of chain N is emitted after stage 1 of
            # chain N+1 so its rope-swap matmul (which waits on DVE work)
            # never blocks the in-order PE queue.
            chain_state = {}

            def chain_s1(oc, qs):
                w = load_wqk(oc)
                sl = slice(qs * 512, (qs + 1) * 512)
                ps = spare.tile([128, 512], f32, tag="ps", name="psqk")
                for c in range(4):
                    nc.tensor.matmul(ps[:, :], w[:, c, :], xT_sb[:, c, sl],
                                     start=(c == 0), stop=False)
                yield
                for c in range(4, KC):
                    nc.tensor.matmul(ps[:, :], w[:, c, :], xT_sb[:, c, sl],
                                     start=False, stop=(c == KC - 1))
                raw = ropep.tile([128, 512], f32, tag="raw", name="raw")
                nc.vector.tensor_scalar_add(raw[:, :], ps[:, :],
                                            bqk_sb[:, oc:oc + 1])
                tt = ropep.tile([128, 512], f32r, tag="tt", name="tt")
                nc.vector.tensor_tensor(tt[:, :], raw[:, :], sink_sb[:, sl], MUL)
                chain_state[(oc, qs)] = (raw, tt)
                yield

            def chain_s2(oc, qs):
                raw, tt = chain_state.pop((oc, qs))
                sl = slice(qs * 512, (qs + 1) * 512)
                sw = spare.tile([128, 512], f32, tag="ps", name="pssw")
                nc.tensor.matmul(sw[:, :], pswap_sb[:, :], tt[:, :],
                                 start=True, stop=True)
                cc = ropep.tile([128, 512], f32, tag="cc", name="cc")
                nc.gpsimd.tensor_tensor(cc[:, :], raw[:, :], cosk_sb[:, sl], MUL)
                dst = qTr[:, oc, sl] if oc < 4 else kTr[:, oc - 4, sl]
                nc.vector.tensor_tensor(dst, cc[:, :], sw[:, :], ADD)
                yield

            def vgroup_p0(sb):
                """v projection for head pair 0 (128 cols) of seq block sb."""
                sbc = slice(sb * 128, (sb + 1) * 128)
                ps = spare.tile([128, 512], f32, tag="ps", name="psv0")
                for c in range(KC):
                    nc.tensor.matmul(
                        ps[:, 0:128], xT_sb[:, c, sbc], wv_sb[:, c, 0:128],
                        start=(c == 0), stop=(c == KC - 1),
                    )
                nc.vector.tensor_copy(
                    v_sb[:, sb, 0:2, 0:DH],
                    ps[:, 0:128].rearrange("p (h d) -> p h d", h=2),
                )

            def vrest_gen(sb):
                """v projection for head pairs 1-3 (384 cols) of block sb."""
                sbc = slice(sb * 128, (sb + 1) * 128)
                ps = spare.tile([128, 512], f32, tag="ps", name="psvr")
                for c in range(KC):
                    nc.tensor.matmul(
                        ps[:, 0:384], xT_sb[:, c, sbc], wv_sb[:, c, 128:512],
                        start=(c == 0), stop=(c == KC - 1),
                    )
                    if c == 3:
                        yield
                nc.vector.tensor_copy(
                    v_sb[:, sb, 2:8, 0:DH],
                    ps[:, 0:384].rearrange("p (h d) -> p h d", h=6),
                )
                yield

            def wout_load_gen():
                nc.sync.dma_start(wout_sb[:, :, :, :], wout_d[:, :, :, :])
                yield

            dg_state = {}

            def dgroup_s1(oc, qs):
                sl = slice(qs * 512, (qs + 1) * 512)
                ps = spare.tile([128, 512], f32, tag="ps", name="pso")
                for c in range(KCD):
                    nc.tensor.matmul(
                        ps[:, :], wout_sb[:, oc, c, :], ctxT[:, c, sl],
                        start=(c == 0), stop=(c == KCD - 1),
                    )
                dg_state[(oc, qs)] = ps
                yield

            def dgroup_s2(oc, qs):
                ps = dg_state.pop((oc, qs))
                sl = slice(qs * 512, (qs + 1) * 512)
                ob = obp.tile([128, 512], bf16, tag="ob", name="ob")
                nc.vector.tensor_copy(ob[:, :], ps[:, :])
                nc.sync.dma_start(out_d[oc, :, sl], ob[:, :])
                yield

            # ---------------- filler schedule ----------------
            # (ready, deadline, generator), queue roughly deadline-sorted.
            # pump() advances the head between attention blocks but never
            # starts an item before `ready` (so a far-future dep can't park
            # in the in-order engine queues).  drain_due() force-emits every
            # item whose deadline has arrived BEFORE the consuming iteration
            # is emitted - program order is semantic order in Tile, so a
            # consumer emitted before its producer would read garbage.
            # Chains appear as interleaved (s1 of next, s2 of previous)
            # pairs; vrest items are sprinkled between pairs.
            chain_order = (
                [(0, qs, 0, qs) for qs in range(1, NQS)]          # q(p0)
                + [(0, 4, 5, qs) for qs in range(NQS)]            # k(p1)
                + [(0, 4 + qs, 1, qs) for qs in range(NQS)]       # q(p1)
                + [(3, 8, 6, qs) for qs in range(NQS)]            # k(p2)
                + [(3, 8 + qs, 2, qs) for qs in range(NQS)]       # q(p2)
                + [(7, 12, 7, qs) for qs in range(NQS)]           # k(p3)
                + [(7, 12 + qs, 3, qs) for qs in range(NQS)]      # q(p3)
            )
            vrest_items = deque((1, 4, vrest_gen(sb)) for sb in range(16))
            fillers = deque()
            prev = None
            for (r, dl, oc, qs) in chain_order:
                fillers.append((r, dl, chain_s1(oc, qs)))
                if prev is not None:
                    pr_, pdl_, poc, pqs = prev
                    fillers.append((pr_, pdl_, chain_s2(poc, pqs)))
                prev = (r, dl, oc, qs)
                if vrest_items and dl <= 8:
                    fillers.append(vrest_items.popleft())
            pr_, pdl_, poc, pqs = prev
            fillers.append((pr_, pdl_, chain_s2(poc, pqs)))
            while vrest_items:
                fillers.append(vrest_items.popleft())
            fillers.append((8, 13, wout_load_gen()))
            dprev = None
            for qs in range(NQS):             # out-proj after C(p3, sp=qs)
                for oc in range(8):
                    fillers.append((13 + qs, 99, dgroup_s1(oc, qs)))
                    if dprev is not None:
                        fillers.append((13 + qs, 99, dgroup_s2(*dprev)))
                    dprev = (oc, qs)
            fillers.append((16, 99, dgroup_s2(*dprev)))

            cur_iter = [0]

            def pump(n):
                while n > 0 and fillers:
                    ready, _, gen = fillers[0]
                    if ready > cur_iter[0]:
                        return
                    try:
                        next(gen)
                        n -= 1
                    except StopIteration:
                        fillers.popleft()

            def drain_due(it):
                # run-to-completion every queued item whose deadline has
                # arrived, even behind later-deadline items (same-chain s1
                # precedes s2 in queue order, so intra-chain order holds)
                keep = []
                for (r, dl, gen) in fillers:
                    if dl <= it:
                        for _ in gen:
                            pass
                    else:
                        keep.append((r, dl, gen))
                fillers.clear()
                fillers.extend(keep)

            # ---------------- preamble: k(p0) span 0 + q(p0) span 0; the
            # remaining k(p0) spans are emitted inline in iteration 0 right
            # before the score blocks that consume them, pacing with the
            # xT DMA stream instead of serializing the full load up front.
            for _ in chain_s1(4, 0):
                pass
            for _ in chain_s1(0, 0):
                pass
            for _ in chain_s2(4, 0):
                pass
            for _ in chain_s2(0, 0):
                pass
            iter0_q = deque()
            for qs in range(1, NQS):
                iter0_q.append((4 * qs - 1, chain_s1(4, qs)))
                iter0_q.append((4 * qs - 1, chain_s2(4, qs)))

            def pump0(blk):
                # drain everything due before this block, else one step
                while iter0_q and iter0_q[0][0] < blk:
                    for _ in iter0_q.popleft()[1]:
                        pass
                if iter0_q:
                    try:
                        next(iter0_q[0][1])
                    except StopIteration:
                        iter0_q.popleft()

            # ---------------- fused attention stream ----------------
            for pr in range(PRS):
                for sp in range(NQS):
                    it = pr * NQS + sp
                    cur_iter[0] = it
                    drain_due(it)
                    s1 = slice(sp * 512, (sp + 1) * 512)
                    cE = psCtx.tile([128, 512], f32, tag="ce", name="cE")
                    cO = psCtx.tile([128, 512], f32, tag="co", name="cO")
                    for blk in range(16):
                        if it == 0:
                            pump0(blk)
                        sc = psSc.tile([128, 2, 512], f32, tag="sc", name="sc")
                        for par in range(2):
                            rs = par * 64
                            nc.tensor.matmul(
                                sc[:, par, :],
                                kTr[rs:rs + 64, pr, blk * 128:(blk + 1) * 128],
                                qTr[rs:rs + 64, pr, s1],
                                start=True, stop=True,
                            )
                        et = expp.tile([128, 2, 512], bf16, tag="et", name="et")
                        nc.scalar.activation(et[:, :, :], sc[:, :, :], Exp,
                                             scale=0.125)
                        if it == 0:
                            vgroup_p0(blk)
                        else:
                            pump(2)
                        st, sp_ = (blk == 0), (blk == 15)
                        nc.tensor.matmul(
                            cE[0:DH + 1, :], v_sb[:, blk, 2 * pr, :],
                            et[:, 0, :], start=st, stop=sp_,
                        )
                        nc.tensor.matmul(
                            cO[0:DH + 1, :], v_sb[:, blk, 2 * pr + 1, :],
                            et[:, 1, :], start=st, stop=sp_,
                        )
                    # epilogue part 1: drain ctx accumulators to SBUF fast
                    # so the single-buffered PSUM banks free immediately.
                    ctf = scrp.tile([128, 2, 512], f32, tag="ctf", name="ctf")
                    nc.vector.tensor_copy(ctf[0:DH + 1, 0, :], cE[0:DH + 1, :])
                    nc.vector.tensor_copy(ctf[0:DH + 1, 1, :], cO[0:DH + 1, :])
                    # sums row (partition 64) -> partition 0 via DMA: the
                    # broadcast/recip ops act on tensor partition 0 only.
                    scr2 = scrp.tile([1, 2, 512], f32, tag="scr2", name="scr2")
                    nc.sync.dma_start(scr2[0:1, :, :], ctf[DH:DH + 1, :, :])
                    pump(3)
                    # epilogue part 2: normalize off the critical path.
                    bcs = scrp.tile([64, 2, 512], f32, tag="bcs", name="bcs")
                    nc.gpsimd.partition_broadcast(bcs[0:DH, :, :],
                                                  scr2[0:1, :, :])
                    bc = scrp.tile([64, 2, 512], f32, tag="bc", name="bc")
                    nc.vector.reciprocal_approx_fast(bc[0:DH, :, :],
                                                     bcs[0:DH, :, :])
                    nc.vector.tensor_tensor(ctxT[0:DH, pr, s1],
                                            ctf[0:DH, 0, :], bc[0:DH, 0, :],
                                            MUL)
                    tmp = scrp.tile([64, 512], bf16, tag="tmp", name="tmp")
                    nc.vector.tensor_tensor(tmp[:, :], ctf[0:DH, 1, :],
                                            bc[0:DH, 1, :], MUL)
                    nc.sync.dma_start(ctxT[DH:128, pr, s1], tmp[:, :])
                    pump(3)

            # ---------------- drain remaining fillers ----------------
            cur_iter[0] = 99
            while fillers:
                pump(100)

            if debug:
                nc.sync.dma_start(dq_d[:, :, :], qTr[:, :, :])
                nc.sync.dma_start(dk_d[:, :, :], kTr[:, :, :])
                nc.sync.dma_start(dv_d[:, :, :, :], v_sb[:, :, :, :])
                nc.sync.dma_start(dctx_d[:, :, :], ctxT[:, :, :])

    nc.finalize()
    return nc


def _host_prep(positions, hidden_states, Wqkv, bqkv, Wout, bout):
    import ml_dtypes

    bf16 = ml_dtypes.bfloat16
    hidden_states = np.asarray(hidden_states, dtype=np.float32)
    Wqkv = np.asarray(Wqkv, dtype=np.float32)
    bqkv = np.asarray(bqkv, dtype=np.float32)
    Wout = np.asarray(Wout, dtype=np.float32)
    positions = np.asarray(positions)

    pswap = np.zeros((128, 128), dtype=np.float32)
    for m in range(128):
        if m % DH < HALF:
            pswap[m + HALF, m] = -1.0
        else:
            pswap[m - HALF, m] = 1.0

    inv_freq = 1.0 / (ROPE_BASE ** (np.arange(HALF, dtype=np.float64) / HALF))
    rowmap = np.arange(128) % HALF
    freqs = positions.astype(np.float64)[:, None] * inv_freq[None, :]  # [S, 32]
    cosk = np.ascontiguousarray(np.cos(freqs).astype(np.float32)[:, rowmap].T)
    sink = np.ascontiguousarray(np.sin(freqs).astype(np.float32)[:, rowmap].T)

    xTs = []
    for b in range(B):
        # [128, KC, S]: partition-major so one DMA covers all chunks
        xTs.append(np.ascontiguousarray(
            hidden_states[b].T.reshape(KC, 128, S).transpose(1, 0, 2)
        ).astype(bf16))

    per_hh = []
    for hh in range(2):
        qsl = slice(hh * 512, (hh + 1) * 512)
        ksl = slice(H + hh * 512, H + (hh + 1) * 512)
        vsl = slice(2 * H + hh * 512, 2 * H + (hh + 1) * 512)
        wq = np.ascontiguousarray(
            Wqkv[:, qsl].reshape(KC, 128, 4, 128).transpose(2, 1, 0, 3))
        wk = np.ascontiguousarray(
            Wqkv[:, ksl].reshape(KC, 128, 4, 128).transpose(2, 1, 0, 3))
        wqk = np.concatenate([wq, wk], axis=0).astype(bf16)  # [8,128,KC,128]
        wv = np.ascontiguousarray(
            Wqkv[:, vsl].reshape(KC, 128, 512).transpose(1, 0, 2)).astype(bf16)
        wout = np.ascontiguousarray(
            Wout[hh * 512:(hh + 1) * 512, :]
            .reshape(KCD, 128, 8, 128).transpose(1, 2, 0, 3)).astype(bf16)
        bq = bqkv[:H][qsl].reshape(4, 128).T
        bk = bqkv[H:2 * H][hh * 512:(hh + 1) * 512].reshape(4, 128).T
        bqk = np.ascontiguousarray(np.concatenate([bq, bk], axis=1))  # [128,8]
        per_hh.append((wqk, wv, wout, bqk))

    in_maps = []
    for c in range(N_CORES):
        b, hh = c // 2, c % 2
        wqk, wv, wout, bqk = per_hh[hh]
        in_maps.append({
            "xT": xTs[b], "wqk": wqk, "wv": wv, "wout": wout,
            "pswap": pswap, "cosk": cosk, "sink": sink,
            "bqk": bqk,
        })
    return in_maps


def kernel(positions, hidden_states, Wqkv, bqkv, Wout, bout):
    global _nc_cache, last_results
    from concourse import bass_utils

    if _nc_cache is None:
        _nc_cache = _build_nc()
    nc = _nc_cache

    in_maps = _host_prep(positions, hidden_states, Wqkv, bqkv, Wout, bout)
    res = bass_utils.run_bass_kernel_spmd(
        nc, in_maps, core_ids=list(range(N_CORES)),
        trace=bool(int(os.environ.get("KERNEL_TRACE", "0") or "0")),
    )
    last_results = res

    bqkv = np.asarray(bqkv, dtype=np.float32)
    Wout = np.asarray(Wout, dtype=np.float32)
    bout = np.asarray(bout, dtype=np.float32)
    # v-bias contribution (attn rows sum to 1) + output bias, added on host
    bias_full = (bout + bqkv[2 * H:].astype(np.float64) @
                 Wout.astype(np.float64)).astype(np.float32)

    out = np.empty((B, S, H), dtype=np.float32)
    for b in range(B):
        p0 = np.asarray(res.results[2 * b]["outT"]).astype(np.float32)
        p1 = np.asarray(res.results[2 * b + 1]["outT"]).astype(np.float32)
        o = (p0 + p1).reshape(H, S)
        out[b] = o.T + bias_full[None, :]
    return out
